# revision 30
# baseline (speedup 1.0000x reference)
"""Trainium2 Bass kernel for nn_BaseConv_137438953680.

Computation (per reference):
  h  = silu(causal_dwconv(u, w1, b1))       # k=3 depthwise
  v  = causal_dwconv(h, w2, b2)             # k=128 depthwise
  p  = silu(u @ Wp.T + bp)                  # square projection
  y  = v * p

Sharding: data-parallel over (batch, half-length) -> 8 chunks of 2048
timesteps, one per NeuronCore. Causal halo (256 steps) is materialized
host-side (zero-padded at batch starts).

The wall-clock of kernel() is dominated by the axon tunnel transfers
(~115 MB/s up, ~37 MB/s down, strictly serial), not device compute, so
everything is organized to minimize bytes moved and host-side work:
  - u is uploaded TIME-MAJOR as bf16 (contiguous slices, no host transpose)
    and transposed to channel-major on device via PE into a DRAM bounce.
  - y is returned as bf16 and upconverted on host (bit-twiddling numpy).
  - The large constants (WpT 4MB, Cs 2MB, Fm, Mi) stay f32 for precision
    but are sharded 8-ways across cores and AllGathered on device over
    NeuronLink; the host-side "concat of shards" is just the full matrix,
    so they are passed to PJRT with zero host copies.
  - A custom PJRT runner (mirroring bass2jax.run_bass_via_pjrt) caches the
    jit callable and skips per-core concat entirely.

Per-core mapping (unchanged from the f32 baseline):
  - conv1: channel-major on VectorE from the device-transposed uT (shifts =
    free-axis offsets, per-channel weights = per-partition scalars), SiLU on
    ScalarE.
  - h transposed to time-major via TensorE tile transposes.
  - conv2: overlap-save spectral method. 256-pt real DFT as matmuls with
    shared DFT matrices; per-channel spectral multiply on VectorE; inverse
    DFT as matmuls.
  - GEMM u @ Wp.T: TensorE, lhsT = uT tiles, rhs = pretransposed WpT,
    bias via a rank-1 (K=1) accumulating matmul, SiLU+PSUM-drain on ScalarE.
  - final elementwise multiply on VectorE, bf16 output.
"""
import sys

sys.path.insert(0, "/opt/trn_rl_repo")

from concurrent.futures import ThreadPoolExecutor

import numpy as np
import ml_dtypes
import jax
import jax.numpy as jnp
from jax.sharding import Mesh, PartitionSpec
from jax.experimental.shard_map import shard_map
import concourse.bass as bass
import concourse.mybir as mybir
import concourse.bacc as bacc
import concourse.tile as tile
from concourse.bass2jax import (
    _bass_exec_p, partition_id_tensor, install_neuronx_cc_hook)

B, L, D = 4, 4096, 1024
NCORES = 8
HOP = 128
NFFT = 256
HALO = 256          # u halo steps (>=130 needed; 2 full tiles for alignment)
NB_FULL = 16        # output blocks of 128 per core (16*128 = 2048)
KD = D // 128       # 8 d-tiles
T_CORE = (B * L) // NCORES
W_IN = HALO + T_CORE               # 2304 u rows per core
NW = W_IN // 128                   # 18 time-major u tiles per core

MM_DT = mybir.dt.float32
BF = ml_dtypes.bfloat16
U_INT8 = True      # upload u as int8 + per-row scale instead of bf16
Y_INT8 = True      # download y as int8 + per-8-channel-group bf16 scales
YGRP = 8           # channels per quantization group
QMAX = 126.0       # < 127 so bf16 round-down of the scale can't overflow int8

_nc_cache: dict = {}
_pool = ThreadPoolExecutor(NCORES)
_bufs: dict = {}

# hmask: zero the h history tile at batch starts (cores 0,2,4,6)
_HMASK = np.repeat(np.array([0.0, 1.0] * B, np.float32)[:, None], 128, axis=0
                   ).reshape(NCORES * 128, 1).copy()


def f32_to_bf16(a):
    """Round-to-nearest-even f32 -> bf16 via bit twiddling (fast)."""
    x = np.ascontiguousarray(a, np.float32).view(np.uint32)
    r = x + np.uint32(0x7FFF) + ((x >> np.uint32(16)) & np.uint32(1))
    return (r >> np.uint32(16)).astype(np.uint16).view(BF)


def bf16_to_f32(v, out):
    u32 = np.asarray(v).view(np.uint16).astype(np.uint32)
    u32 <<= np.uint32(16)
    out[...] = u32.view(np.float32)


# ---------------------------------------------------------------- host consts
def _dft_consts():
    """Forward/inverse real-DFT matrices, packed for SBUF tiles (f32)."""
    s = np.arange(NFFT)
    F = np.zeros((NFFT, NFFT))  # [sample, row] rows: 0..128 Re, 129..255 Im
    for k in range(129):
        F[:, k] = np.cos(2 * np.pi * k * s / NFFT)
    for k in range(1, 128):
        F[:, 128 + k] = -np.sin(2 * np.pi * k * s / NFFT)
    M = np.zeros((NFFT, HOP))  # [row, m-128]
    for mi in range(HOP):
        m = 128 + mi
        M[0, mi] = 1.0 / NFFT
        M[128, mi] = ((-1) ** m) / NFFT
        for k in range(1, 128):
            M[k, mi] = 2.0 * np.cos(2 * np.pi * k * m / NFFT) / NFFT
            M[128 + k, mi] = -2.0 * np.sin(2 * np.pi * k * m / NFFT) / NFFT
    Fm = np.zeros((128, 512), dtype=np.float32)
    for st in range(2):
        for bt in range(2):
            Fm[:, (st * 2 + bt) * 128:(st * 2 + bt + 1) * 128] = \
                F[st * 128:(st + 1) * 128, bt * 128:(bt + 1) * 128]
    Mi = np.zeros((128, 256), dtype=np.float32)
    for kt in range(2):
        Mi[:, kt * 128:(kt + 1) * 128] = M[kt * 128:(kt + 1) * 128, :]
    return Fm, Mi


def _spectral_weights(w2):
    """Pointwise coefficient tiles C0..C3, each [128, D], packed [128, 4D]."""
    d = w2.shape[1]
    f = np.zeros((NFFT, d))
    f[:128] = w2[::-1, :]
    Fh = np.fft.rfft(f, n=NFFT, axis=0)      # rows 0..128 of the DFT
    Fr, Fi = Fh.real, Fh.imag
    C0 = Fr[0:128].copy()
    C1 = np.zeros((128, d)); C1[1:] = -Fi[1:128]
    C2 = np.empty((128, d)); C2[0] = Fr[128]; C2[1:] = Fr[1:128]
    C3 = np.zeros((128, d)); C3[1:] = Fi[1:128]
    return np.concatenate([C0, C1, C2, C3], axis=1).astype(np.float32)  # [128, 4*D]


def host_consts(w1, b1, w2, b2, Wp, bp):
    w1r = np.asarray(w1, np.float64)[:, 0, :]   # (3, D)
    w2r = np.asarray(w2, np.float64)[:, 0, :]   # (128, D)
    Fm, Mi = _dft_consts()
    Cs = _spectral_weights(w2r)
    w1s = np.zeros((128, 3 * KD), dtype=np.float32)
    b1s = np.zeros((128, KD), dtype=np.float32)
    for k in range(KD):
        for j in range(3):
            w1s[:, j * KD + k] = w1r[j, k * 128:(k + 1) * 128]
        b1s[:, k] = np.asarray(b1, np.float64)[k * 128:(k + 1) * 128]
    WpT = np.ascontiguousarray(np.asarray(Wp, np.float32).T)      # [D, D]
    b2r = (NFFT * np.asarray(b2, np.float64)).astype(np.float32)[None, :]  # [1, D]
    bp1 = np.asarray(bp, np.float32)[None, :]                     # [1, D]
    eye = np.eye(128, dtype=np.float32)
    return dict(Fm=Fm, Minv=Mi, Cs=Cs, w1s=w1s, b1s=b1s, WpT=WpT,
                b2r=b2r, bp1=bp1, eye=eye)


def make_global_inputs(u, consts, put=None):
    """Global (already core-concatenated) input arrays for the PJRT runner.

    `put` (optional) asynchronously device_puts the constant arrays while the
    host threads quantize u, hiding the constants' upload entirely.
    """
    u = np.asarray(u, np.float32)
    g = {
        # shard-concats of the big consts are just the full matrices
        "WpT_sh": consts["WpT"], "Cs_sh": consts["Cs"],
        "Fm_sh": consts["Fm"], "Mi_sh": consts["Minv"],
        # small per-core consts, replicated
        "w1s": np.tile(consts["w1s"], (NCORES, 1)),
        "b1s": np.tile(consts["b1s"], (NCORES, 1)),
        "b2r": np.tile(consts["b2r"], (NCORES, 1)),
        "bp1": np.tile(consts["bp1"], (NCORES, 1)),
        "eye": np.tile(consts["eye"], (NCORES, 1)),
        "hmask": _HMASK,
    }
    if put is not None:
        g = {k: put(v) for k, v in g.items()}

    uf = u.reshape(B * L, D)
    if U_INT8:
        # quantize per core-band directly into the global buffer with a
        # small reused temp; odd cores' halos are copied from the previous
        # core's tail (same u rows, same quantization)
        if not _bufs:
            _bufs["t"] = np.empty((T_CORE, D), np.float32)
            _bufs["gu"] = np.zeros((NCORES * W_IN, D), np.int8)
            _bufs["gs"] = np.zeros((NCORES * W_IN, 1), np.float32)
        tbuf = _bufs["t"]
        g_u, g_us = _bufs["gu"], _bufs["gs"]
        for ci in range(NCORES):
            seg = uf[ci * T_CORE:(ci + 1) * T_CORE]
            rowmax = np.maximum(seg.max(axis=1), -seg.min(axis=1))
            inv = np.where(rowmax > 0, 127.0 / np.maximum(rowmax, 1e-30), 0.0)
            np.multiply(seg, inv[:, None], out=tbuf)
            np.rint(tbuf, out=tbuf)
            r0 = ci * W_IN + HALO
            g_u[r0:r0 + T_CORE] = tbuf
            g_us[r0:r0 + T_CORE, 0] = rowmax * (1.0 / 127.0)
        for ci in range(1, NCORES, 2):       # halo for mid-batch cores
            r0 = ci * W_IN
            g_u[r0:r0 + HALO] = g_u[r0 - HALO:r0]
            g_us[r0:r0 + HALO] = g_us[r0 - HALO:r0]
        g["u_tm"] = g_u
        g["u_sc"] = g_us
    else:
        g_u = np.zeros((NCORES * W_IN, D), BF)

        def mk_chunk(ci):
            bi, half = divmod(ci, NCORES // B)
            t0 = bi * L + half * T_CORE
            lo = max(bi * L, t0 - HALO)
            r0 = ci * W_IN + HALO - (t0 - lo)
            n = t0 + T_CORE - lo
            g_u[r0:r0 + n] = f32_to_bf16(uf[lo:lo + n])

        list(_pool.map(mk_chunk, range(NCORES)))
        g["u_tm"] = g_u
    return g


# ---------------------------------------------------------------- bass build
def build_nc(n_blocks=NB_FULL, mm_dt=MM_DT, reps=1):
    T = n_blocks * HOP
    W = HALO + T                       # uT width (2304 for full problem)
    nw = W // 128
    nc = bacc.Bacc("TRN2", target_bir_lowering=False, debug=False,
                   num_devices=NCORES)
    f32 = mybir.dt.float32
    bf16 = mybir.dt.bfloat16

    if U_INT8:
        u_d = nc.dram_tensor("u_tm", [W, D], mybir.dt.int8,
                             kind="ExternalInput").ap()
        us_d = nc.dram_tensor("u_sc", [W, 1], f32, kind="ExternalInput").ap()
    else:
        u_d = nc.dram_tensor("u_tm", [W, D], bf16, kind="ExternalInput").ap()
    WpT_d = nc.dram_tensor("WpT_sh", [128, D], f32, kind="ExternalInput").ap()
    Cs_d = nc.dram_tensor("Cs_sh", [16, 4 * D], f32, kind="ExternalInput").ap()
    Fm_d = nc.dram_tensor("Fm_sh", [16, 512], f32, kind="ExternalInput").ap()
    Mi_d = nc.dram_tensor("Mi_sh", [16, 256], f32, kind="ExternalInput").ap()
    w1s_d = nc.dram_tensor("w1s", [128, 3 * KD], f32, kind="ExternalInput").ap()
    b1s_d = nc.dram_tensor("b1s", [128, KD], f32, kind="ExternalInput").ap()
    b2r_d = nc.dram_tensor("b2r", [1, D], f32, kind="ExternalInput").ap()
    bp1_d = nc.dram_tensor("bp1", [1, D], f32, kind="ExternalInput").ap()
    eye_d = nc.dram_tensor("eye", [128, 128], f32, kind="ExternalInput").ap()
    hm_d = nc.dram_tensor("hmask", [128, 1], f32, kind="ExternalInput").ap()
    if Y_INT8:
        NG = D // YGRP
        yq_d = nc.dram_tensor("y_q", [T, D], mybir.dt.int8,
                              kind="ExternalOutput").ap()
        ys_d = nc.dram_tensor("y_s", [T, NG], bf16, kind="ExternalOutput").ap()
    else:
        y_d = nc.dram_tensor("y", [T, D], bf16, kind="ExternalOutput").ap()

    RG = [list(range(NCORES))]
    BYPASS = mybir.AluOpType.bypass

    from contextlib import ExitStack
    with tile.TileContext(nc) as tc, ExitStack() as ctx:
        dramp = ctx.enter_context(tc.tile_pool(name="ccdram", bufs=1,
                                               space="DRAM"))
        cpool = ctx.enter_context(tc.tile_pool(name="consts", bufs=1))

        # ---- AllGather the sharded constants over NeuronLink
        def gather(src_ap, rows, cols):
            bin_ = dramp.tile([rows, cols], f32)
            bout = dramp.tile([rows * NCORES, cols], f32)
            nc.gpsimd.dma_start(bin_[:], src_ap)
            nc.gpsimd.collective_compute(
                "AllGather", BYPASS, replica_groups=RG,
                ins=[bin_[:].opt()], outs=[bout[:].opt()])
            return bout

        wpt_g = gather(WpT_d[:], 128, D)        # [1024, 1024] = WpT
        cs_g = gather(Cs_d[:], 16, 4 * D)       # [128, 4096]
        fm_g = gather(Fm_d[:], 16, 512)         # [128, 512]
        mi_g = gather(Mi_d[:], 16, 256)         # [128, 256]

        # resident constants in SBUF
        wpt = cpool.tile([128, KD * D], f32, tag="wpt")
        for k in range(KD):
            nc.sync.dma_start(wpt[:, k * D:(k + 1) * D],
                              wpt_g[k * 128:(k + 1) * 128, :])
        fm = cpool.tile([128, 512], f32, tag="fm")
        nc.sync.dma_start(fm[:], fm_g[:])
        mi = cpool.tile([128, 256], f32, tag="mi")
        nc.sync.dma_start(mi[:], mi_g[:])
        cs = cpool.tile([128, 4 * D], f32, tag="cs")
        nc.sync.dma_start(cs[:], cs_g[:])
        w1s = cpool.tile([128, 3 * KD], f32, tag="w1s")
        nc.sync.dma_start(w1s[:], w1s_d[:])
        b1s = cpool.tile([128, KD], f32, tag="b1s")
        nc.sync.dma_start(b1s[:], b1s_d[:])
        b2r = cpool.tile([1, D], f32, tag="b2r")
        nc.sync.dma_start(b2r[:], b2r_d[:])
        bp1 = cpool.tile([1, D], f32, tag="bp1")
        nc.sync.dma_start(bp1[:], bp1_d[:])
        eye = cpool.tile([128, 128], f32, tag="eye")
        nc.sync.dma_start(eye[:], eye_d[:])
        hm = cpool.tile([128, 1], f32, tag="hm")
        nc.sync.dma_start(hm[:], hm_d[:])
        ones1 = cpool.tile([1, 128], f32, tag="ones1")
        nc.gpsimd.memset(ones1[:], 1.0)
        if not U_INT8:
            eye_bf = cpool.tile([128, 128], bf16, tag="eye_bf")
            nc.vector.tensor_copy(eye_bf[:], eye[:])

        # DRAM bounce for the channel-major u (written by the PE transposes)
        uT_b = dramp.tile([D, W], f32)
        uT3 = uT_b[:].rearrange("(k p) t -> p k t", p=128)

        utm_p = ctx.enter_context(tc.tile_pool(name="utm", bufs=3))
        upool = ctx.enter_context(tc.tile_pool(name="uq", bufs=3))
        scr = ctx.enter_context(tc.tile_pool(name="scr", bufs=6))
        hcm_p = ctx.enter_context(tc.tile_pool(name="hcm", bufs=2))
        hsb_p = ctx.enter_context(tc.tile_pool(name="hsb", bufs=3))
        yt_p = ctx.enter_context(tc.tile_pool(name="yt", bufs=4))
        psb_p = ctx.enter_context(tc.tile_pool(name="psb", bufs=4))
        ysb_p = ctx.enter_context(tc.tile_pool(name="ysb", bufs=2))
        gms_p = ctx.enter_context(tc.tile_pool(name="gms", bufs=2))

        htr_p = ctx.enter_context(tc.tile_pool(name="htr", bufs=1, space="PSUM"))
        xps_p = ctx.enter_context(tc.tile_pool(name="xps", bufs=1, space="PSUM"))
        vps_p = ctx.enter_context(tc.tile_pool(name="vps", bufs=2, space="PSUM"))
        pps_p = ctx.enter_context(tc.tile_pool(name="pps", bufs=2, space="PSUM"))

        MULT = mybir.AluOpType.mult
        ADD = mybir.AluOpType.add
        SILU = mybir.ActivationFunctionType.Silu
        COPY = mybir.ActivationFunctionType.Copy

        # ---- preamble: transpose time-major u into channel-major DRAM bounce
        # (PSUM from vps_p is free until the main loop's first IDFT)
        for w in range(nw):
            if U_INT8:
                ui = utm_p.tile([128, D], mybir.dt.int8, tag="utm8", bufs=2)
                nc.sync.dma_start(ui[:], u_d[w * 128:(w + 1) * 128, :])
                usc = scr.tile([128, 1], f32, tag="usc", bufs=2)
                nc.sync.dma_start(usc[:], us_d[w * 128:(w + 1) * 128, :])
                uf = utm_p.tile([128, D], f32, tag="utmf", bufs=2)
                nc.scalar.activation(uf[:], ui[:], COPY)
                ut = utm_p.tile([128, D], f32, tag="utms", bufs=2)
                nc.vector.tensor_scalar_mul(ut[:], uf[:], usc[:, 0:1])
                teye, tdt = eye, f32
            else:
                ub = utm_p.tile([128, D], bf16, tag="utmb", bufs=2)
                nc.sync.dma_start(ub[:], u_d[w * 128:(w + 1) * 128, :])
                ut = utm_p.tile([128, D], f32, tag="utmf", bufs=2)
                nc.scalar.activation(ut[:], ub[:], COPY)
                teye, tdt = eye, f32
            for g in range(2):
                ps = vps_p.tile([128, 512], f32, tag="vps")
                for j in range(4):
                    k = g * 4 + j
                    nc.tensor.transpose(
                        ps[:, j * 128:(j + 1) * 128],
                        ut[:, k * 128:(k + 1) * 128], teye[:])
                st = scr.tile([128, 512], f32, tag="pre_st", bufs=2)
                nc.scalar.activation(st[:], ps[:], COPY)
                for j in range(4):
                    k = g * 4 + j
                    nc.sync.dma_start(
                        uT_b[k * 128:(k + 1) * 128, w * 128:(w + 1) * 128],
                        st[:, j * 128:(j + 1) * 128])

        def mk_h_tile(hq):
            """conv1 (c-major, DVE+GPS) + silu (ACT) + transpose (PE) to a
            time-major h tile [128(t), D(ch)]."""
            base = HALO + hq * HOP
            uq = upool.tile([128, KD, 130], f32, tag="uq")
            nc.sync.dma_start(uq[:], uT3[:, :, base - 2:base + 128])
            hcm = hcm_p.tile([128, KD * 128], f32, tag="hcm")
            for k in range(KD):
                t1 = scr.tile([128, 128], f32, tag="scr1")
                nc.gpsimd.tensor_scalar(
                    t1[:], uq[:, k, 0:128], w1s[:, 0 * KD + k:0 * KD + k + 1],
                    None, MULT)
                t2 = scr.tile([128, 128], f32, tag="scr2")
                nc.gpsimd.tensor_scalar(
                    t2[:], uq[:, k, 1:129], w1s[:, 1 * KD + k:1 * KD + k + 1],
                    None, MULT)
                t3 = scr.tile([128, 128], f32, tag="scr3")
                nc.gpsimd.tensor_tensor(t3[:], t1[:], t2[:], ADD)
                t4 = scr.tile([128, 128], f32, tag="scr4")
                nc.vector.tensor_scalar(
                    t4[:], uq[:, k, 2:130], w1s[:, 2 * KD + k:2 * KD + k + 1],
                    b1s[:, k:k + 1], MULT, ADD)
                nc.vector.tensor_tensor(
                    hcm[:, k * 128:(k + 1) * 128], t3[:], t4[:], ADD)
            hcm2 = hcm_p.tile([128, KD * 128], f32, tag="hcm2")
            nc.scalar.activation(hcm2[:], hcm[:], SILU)
            htr = htr_p.tile([128, D], f32, tag="htr")
            for k in range(KD):
                nc.tensor.transpose(
                    htr[:, k * 128:(k + 1) * 128],
                    hcm2[:, k * 128:(k + 1) * 128], eye[:])
            hsb = hsb_p.tile([128, D], f32, tag="hsb")
            if hq < 0:
                nc.vector.tensor_scalar_mul(hsb[:], htr[:], hm[:, 0:1])
            else:
                nc.vector.tensor_copy(hsb[:], htr[:])
            return uq, hsb

        from contextlib import nullcontext
        loop_ctx = tc.For_i(0, reps, 1) if reps > 1 else nullcontext()
        with loop_ctx:
            h_tiles: dict = {}
            uq_tiles: dict = {}
            uq_tiles[-1], h_tiles[-1] = mk_h_tile(-1)
            uq_tiles[0], h_tiles[0] = mk_h_tile(0)
            for q in range(n_blocks):
                uq = uq_tiles.pop(q)
                hsb = h_tiles[q]
                hprev = h_tiles.pop(q - 1)
                ysb = ysb_p.tile([128, D], f32 if Y_INT8 else bf16, tag="ysb")
                # ---- GEMM both halves (PE work first; only needs uq + consts)
                pps_t = []
                for half in range(2):
                    e0 = half * 512
                    pps = pps_p.tile([128, 512], f32, tag="pps")
                    for k in range(KD):
                        nc.tensor.matmul(
                            pps[:],
                            uq[:, k, 2:130].bitcast(mm_dt),
                            wpt[:, k * D + e0:k * D + e0 + 512].bitcast(mm_dt),
                            start=(k == 0), stop=False)
                    nc.tensor.matmul(
                        pps[:], ones1[:].bitcast(mm_dt),
                        bp1[:, e0:e0 + 512].bitcast(mm_dt),
                        start=False, stop=True)
                    pps_t.append(pps)
                # ---- forward DFT both halves
                x_t = []
                for half in range(2):
                    e0 = half * 512
                    x0 = xps_p.tile([128, 512], f32, tag="xps0")
                    x1 = xps_p.tile([128, 512], f32, tag="xps1")
                    for bt, xps in ((0, x0), (1, x1)):
                        nc.tensor.matmul(
                            xps[:],
                            fm[:, (0 * 2 + bt) * 128:(0 * 2 + bt + 1) * 128].bitcast(mm_dt),
                            hprev[:, e0:e0 + 512].bitcast(mm_dt),
                            start=True, stop=False)
                        nc.tensor.matmul(
                            xps[:],
                            fm[:, (1 * 2 + bt) * 128:(1 * 2 + bt + 1) * 128].bitcast(mm_dt),
                            hsb[:, e0:e0 + 512].bitcast(mm_dt),
                            start=False, stop=True)
                    x_t.append((x0, x1))
                # ---- silu(p) early: frees GEMM PSUM banks a block sooner
                psb_t = []
                for half in range(2):
                    psb = psb_p.tile([128, 512], f32, tag="psb")
                    nc.scalar.activation(psb[:], pps_t[half][:], SILU)
                    psb_t.append(psb)
                # ---- spectral pointwise (DVE muls read PSUM; GPS does adds)
                yt_t = []
                for half in range(2):
                    e0 = half * 512
                    x0, x1 = x_t[half]
                    yt0 = yt_p.tile([128, 512], f32, tag="yt0")
                    yt1 = yt_p.tile([128, 512], f32, tag="yt1")
                    ta = scr.tile([128, 512], f32, tag="scra")
                    tb = scr.tile([128, 512], f32, tag="scrb")
                    nc.vector.tensor_tensor(yt0[:], x0[:], cs[:, 0 * D + e0:0 * D + e0 + 512], MULT)
                    nc.vector.tensor_tensor(ta[:], x1[:], cs[:, 1 * D + e0:1 * D + e0 + 512], MULT)
                    nc.gpsimd.tensor_tensor(yt0[:], yt0[:], ta[:], ADD)
                    nc.vector.tensor_tensor(
                        yt0[0:1, :], yt0[0:1, :], b2r[0:1, e0:e0 + 512], ADD)
                    nc.vector.tensor_tensor(yt1[:], x1[:], cs[:, 2 * D + e0:2 * D + e0 + 512], MULT)
                    nc.vector.tensor_tensor(tb[:], x0[:], cs[:, 3 * D + e0:3 * D + e0 + 512], MULT)
                    nc.gpsimd.tensor_tensor(yt1[:], yt1[:], tb[:], ADD)
                    yt_t.append((yt0, yt1))
                # ---- next block's h (PE transposes slot between DFT and IDFT,
                #      giving DVE/GPS time to finish pointwise)
                if q + 1 < n_blocks:
                    uq_tiles[q + 1], h_tiles[q + 1] = mk_h_tile(q + 1)
                # ---- inverse DFT + final multiply
                for half in range(2):
                    e0 = half * 512
                    yt0, yt1 = yt_t[half]
                    vps = vps_p.tile([128, 512], f32, tag="vps")
                    nc.tensor.matmul(vps[:], mi[:, 0:128].bitcast(mm_dt),
                                     yt0[:].bitcast(mm_dt), start=True, stop=False)
                    nc.tensor.matmul(vps[:], mi[:, 128:256].bitcast(mm_dt),
                                     yt1[:].bitcast(mm_dt), start=False, stop=True)
                    nc.vector.tensor_tensor(
                        ysb[:, e0:e0 + 512], vps[:], psb_t[half][:], MULT)
                if Y_INT8:
                    # quantize: q = rint(y * QMAX / bf16(groupmax)), scale out
                    NG = D // YGRP
                    yf3 = ysb[:].rearrange("p (g j) -> p g j", j=YGRP)
                    gm = gms_p.tile([128, NG], f32, tag="gm")
                    nc.vector.tensor_reduce(gm[:], yf3, mybir.AxisListType.X,
                                            mybir.AluOpType.max,
                                            apply_absolute_value=True)
                    gmb = gms_p.tile([128, NG], bf16, tag="gmb")
                    nc.gpsimd.tensor_copy(gmb[:], gm[:])
                    gmf = gms_p.tile([128, NG], f32, tag="gmf")
                    nc.gpsimd.tensor_copy(gmf[:], gmb[:])
                    inv = gms_p.tile([128, NG], f32, tag="inv")
                    nc.vector.reciprocal(inv[:], gmf[:])
                    invq = gms_p.tile([128, NG], f32, tag="invq")
                    nc.vector.tensor_scalar_mul(invq[:], inv[:], QMAX)
                    yq = ysb_p.tile([128, D], mybir.dt.int8, tag="yq")
                    yq3 = yq[:].rearrange("p (g j) -> p g j", j=YGRP)
                    for j in range(YGRP):
                        nc.vector.tensor_tensor(yq3[:, :, j], yf3[:, :, j],
                                                invq[:], MULT)
                    nc.sync.dma_start(yq_d[q * HOP:(q + 1) * HOP, :], yq[:])
                    nc.sync.dma_start(ys_d[q * HOP:(q + 1) * HOP, :], gmb[:])
                else:
                    nc.sync.dma_start(y_d[q * HOP:(q + 1) * HOP, :], ysb[:])

    nc.compile()
    return nc


# ---------------------------------------------------------------- PJRT runner
def _make_runner(nc):
    """Cached jit callable mirroring bass2jax.run_bass_via_pjrt (multi-core)."""
    install_neuronx_cc_hook()
    partition_name = nc.partition_id_tensor.name if nc.partition_id_tensor else None
    in_names, out_names, out_avals, out_shapes = [], [], [], []
    for alloc in nc.m.functions[0].allocations:
        if not isinstance(alloc, mybir.MemoryLocationSet):
            continue
        name = alloc.memorylocations[0].name
        if alloc.kind == "ExternalInput":
            if name != partition_name:
                in_names.append(name)
        elif alloc.kind == "ExternalOutput":
            out_names.append(name)
            shape = tuple(alloc.tensor_shape)
            dtype = mybir.dt.np(alloc.dtype)
            out_avals.append(jax.core.ShapedArray(shape, dtype))
            out_shapes.append((shape, dtype))
    n_params = len(in_names)
    n_outs = len(out_avals)
    in_names_all = in_names + out_names
    if partition_name is not None:
        in_names_all.append(partition_name)
    donate = tuple(range(n_params, n_params + n_outs))

    def _body(*args):
        operands = list(args)
        if partition_name is not None:
            operands.append(partition_id_tensor())
        outs = _bass_exec_p.bind(
            *operands, out_avals=tuple(out_avals), in_names=tuple(in_names_all),
            out_names=tuple(out_names), lowering_input_output_aliases=(),
            sim_require_finite=True, sim_require_nnan=True, nc=nc)
        return tuple(outs)

    devices = jax.devices()[:NCORES]
    mesh = Mesh(np.asarray(devices), ("core",))
    in_specs = (PartitionSpec("core"),) * (n_params + n_outs)
    out_specs = (PartitionSpec("core"),) * len(out_names)
    # No donation: our NEFF writes every output element, so the zero buffers
    # are only parameter-order placeholders. Keeping them undonated lets us
    # upload them ONCE and reuse the device-resident copies every call.
    sharded = jax.jit(
        shard_map(_body, mesh=mesh, in_specs=in_specs, out_specs=out_specs,
                  check_rep=False),
        keep_unused=True)
    sharding = jax.sharding.NamedSharding(mesh, PartitionSpec("core"))
    dev_zeros: list = []

    def put(x):
        return jax.device_put(x, sharding)

    def run(global_ins: dict):
        if not dev_zeros:
            dev_zeros.append(tuple(
                jax.device_put(np.zeros((NCORES * s[0], *s[1:]), dt), sharding)
                for s, dt in out_shapes))
        args = [global_ins[n] for n in in_names]
        outs = sharded(*args, *dev_zeros[0])
        return {n: np.asarray(outs[i]) for i, n in enumerate(out_names)}

    return run, put


def get_nc(n_blocks=NB_FULL, mm_dt=MM_DT, reps=1):
    key = (n_blocks, str(mm_dt), reps)
    if key not in _nc_cache:
        nc = build_nc(n_blocks, mm_dt, reps)
        run, put = _make_runner(nc)
        _nc_cache[key] = (nc, run, put)
    return _nc_cache[key]


# ---------------------------------------------------------------- entry point
_const_cache: dict = {}


def _weights_key(ws):
    import hashlib
    h = hashlib.blake2b(digest_size=16)
    for w in ws:
        a = np.ascontiguousarray(w)
        h.update(str(a.shape).encode())
        h.update(a.tobytes())
    return h.digest()


def kernel(u, w1, b1, w2, b2, Wp, bp):
    nc, run, put = get_nc()
    key = _weights_key([w1, b1, w2, b2, Wp, bp])
    if key not in _const_cache:
        consts = host_consts(w1, b1, w2, b2, Wp, bp)
        _const_cache.clear()   # keep at most one entry
        _const_cache[key] = consts
        put_consts = put       # device_put the const arrays (cached below)
    else:
        consts = _const_cache[key]
        put_consts = None      # already device-resident jax arrays
    gins = make_global_inputs(u, consts, put=put_consts)
    if put_consts is not None:
        # overwrite the host arrays with their device-resident versions so
        # the next call skips the upload entirely
        for k in ("WpT_sh", "Cs_sh", "Fm_sh", "Mi_sh", "w1s", "b1s", "b2r",
                  "bp1", "eye", "hmask"):
            consts.setdefault("_dev", {})[k] = gins[k]
    else:
        for k, v in consts.get("_dev", {}).items():
            gins[k] = v
    res = run(gins)
    y = np.empty((B, L, D), dtype=np.float32)

    if Y_INT8:
        NG = D // YGRP
        yq = res["y_q"]                # [NCORES*T_CORE, D] int8
        ys = res["y_s"]                # [NCORES*T_CORE, NG] bf16

        def fill(ci):
            bi, half = divmod(ci, NCORES // B)
            sl = slice(ci * T_CORE, (ci + 1) * T_CORE)
            s32 = np.empty((T_CORE, NG), np.float32)
            bf16_to_f32(ys[sl], s32)
            s32 *= np.float32(1.0 / QMAX)
            out = y[bi, half * T_CORE:(half + 1) * T_CORE]
            np.multiply(yq[sl].reshape(T_CORE, NG, YGRP), s32[:, :, None],
                        out=out.reshape(T_CORE, NG, YGRP))
    else:
        yg = res["y"]                  # [NCORES*T_CORE, D] bf16

        def fill(ci):
            bi, half = divmod(ci, NCORES // B)
            bf16_to_f32(yg[ci * T_CORE:(ci + 1) * T_CORE],
                        y[bi, half * T_CORE:(half + 1) * T_CORE])

    list(_pool.map(fill, range(NCORES)))
    return y


# revision 34
# speedup vs baseline: 1.1041x; 1.1041x over previous
"""Trainium2 Bass kernel for nn_BaseConv_137438953680.

Computation (per reference):
  h  = silu(causal_dwconv(u, w1, b1))       # k=3 depthwise
  v  = causal_dwconv(h, w2, b2)             # k=128 depthwise
  p  = silu(u @ Wp.T + bp)                  # square projection
  y  = v * p

Sharding: data-parallel over (batch, half-length) -> 8 chunks of 2048
timesteps, one per NeuronCore. Causal halo (256 steps) is materialized
host-side (zero-padded at batch starts).

The wall-clock of kernel() is dominated by the axon tunnel transfers
(~115 MB/s up, ~37 MB/s down, strictly serial), not device compute, so
everything is organized to minimize bytes moved and host-side work:
  - u is uploaded TIME-MAJOR as bf16 (contiguous slices, no host transpose)
    and transposed to channel-major on device via PE into a DRAM bounce.
  - y is returned as bf16 and upconverted on host (bit-twiddling numpy).
  - The large constants (WpT 4MB, Cs 2MB, Fm, Mi) stay f32 for precision
    but are sharded 8-ways across cores and AllGathered on device over
    NeuronLink; the host-side "concat of shards" is just the full matrix,
    so they are passed to PJRT with zero host copies.
  - A custom PJRT runner (mirroring bass2jax.run_bass_via_pjrt) caches the
    jit callable and skips per-core concat entirely.

Per-core mapping (unchanged from the f32 baseline):
  - conv1: channel-major on VectorE from the device-transposed uT (shifts =
    free-axis offsets, per-channel weights = per-partition scalars), SiLU on
    ScalarE.
  - h transposed to time-major via TensorE tile transposes.
  - conv2: overlap-save spectral method. 256-pt real DFT as matmuls with
    shared DFT matrices; per-channel spectral multiply on VectorE; inverse
    DFT as matmuls.
  - GEMM u @ Wp.T: TensorE, lhsT = uT tiles, rhs = pretransposed WpT,
    bias via a rank-1 (K=1) accumulating matmul, SiLU+PSUM-drain on ScalarE.
  - final elementwise multiply on VectorE, bf16 output.
"""
import sys

sys.path.insert(0, "/opt/trn_rl_repo")

from concurrent.futures import ThreadPoolExecutor

import numpy as np
import ml_dtypes
import jax
import jax.numpy as jnp
from jax.sharding import Mesh, PartitionSpec
from jax.experimental.shard_map import shard_map
import concourse.bass as bass
import concourse.mybir as mybir
import concourse.bacc as bacc
import concourse.tile as tile
from concourse.bass2jax import (
    _bass_exec_p, partition_id_tensor, install_neuronx_cc_hook)

B, L, D = 4, 4096, 1024
NCORES = 8
HOP = 128
NFFT = 256
HALO = 256          # u halo steps (>=130 needed; 2 full tiles for alignment)
NB_FULL = 16        # output blocks of 128 per core (16*128 = 2048)
KD = D // 128       # 8 d-tiles
T_CORE = (B * L) // NCORES
W_IN = HALO + T_CORE               # 2304 u rows per core
NW = W_IN // 128                   # 18 time-major u tiles per core

MM_DT = mybir.dt.float32
BF = ml_dtypes.bfloat16
U_INT8 = True      # upload u as int8 + per-row scale instead of bf16
Y_INT8 = True      # download y as int8 + per-8-channel-group bf16 scales
YGRP = 8           # channels per quantization group
QMAX = 126.0       # < 127 so bf16 round-down of the scale can't overflow int8

_nc_cache: dict = {}
_pool = ThreadPoolExecutor(NCORES)
_bufs: dict = {}

# hmask: zero the h history tile at batch starts (cores 0,2,4,6)
_HMASK = np.repeat(np.array([0.0, 1.0] * B, np.float32)[:, None], 128, axis=0
                   ).reshape(NCORES * 128, 1).copy()


def f32_to_bf16(a):
    """Round-to-nearest-even f32 -> bf16 via bit twiddling (fast)."""
    x = np.ascontiguousarray(a, np.float32).view(np.uint32)
    r = x + np.uint32(0x7FFF) + ((x >> np.uint32(16)) & np.uint32(1))
    return (r >> np.uint32(16)).astype(np.uint16).view(BF)


def bf16_to_f32(v, out):
    u32 = np.asarray(v).view(np.uint16).astype(np.uint32)
    u32 <<= np.uint32(16)
    out[...] = u32.view(np.float32)


# ---------------------------------------------------------------- host consts
def _dft_consts():
    """Forward/inverse real-DFT matrices, packed for SBUF tiles (f32)."""
    s = np.arange(NFFT)
    F = np.zeros((NFFT, NFFT))  # [sample, row] rows: 0..128 Re, 129..255 Im
    for k in range(129):
        F[:, k] = np.cos(2 * np.pi * k * s / NFFT)
    for k in range(1, 128):
        F[:, 128 + k] = -np.sin(2 * np.pi * k * s / NFFT)
    M = np.zeros((NFFT, HOP))  # [row, m-128]
    for mi in range(HOP):
        m = 128 + mi
        M[0, mi] = 1.0 / NFFT
        M[128, mi] = ((-1) ** m) / NFFT
        for k in range(1, 128):
            M[k, mi] = 2.0 * np.cos(2 * np.pi * k * m / NFFT) / NFFT
            M[128 + k, mi] = -2.0 * np.sin(2 * np.pi * k * m / NFFT) / NFFT
    Fm = np.zeros((128, 512), dtype=np.float32)
    for st in range(2):
        for bt in range(2):
            Fm[:, (st * 2 + bt) * 128:(st * 2 + bt + 1) * 128] = \
                F[st * 128:(st + 1) * 128, bt * 128:(bt + 1) * 128]
    Mi = np.zeros((128, 256), dtype=np.float32)
    for kt in range(2):
        Mi[:, kt * 128:(kt + 1) * 128] = M[kt * 128:(kt + 1) * 128, :]
    return Fm, Mi


def _spectral_weights(w2):
    """Pointwise coefficient tiles C0..C3, each [128, D], packed [128, 4D]."""
    d = w2.shape[1]
    f = np.zeros((NFFT, d))
    f[:128] = w2[::-1, :]
    Fh = np.fft.rfft(f, n=NFFT, axis=0)      # rows 0..128 of the DFT
    Fr, Fi = Fh.real, Fh.imag
    C0 = Fr[0:128].copy()
    C1 = np.zeros((128, d)); C1[1:] = -Fi[1:128]
    C2 = np.empty((128, d)); C2[0] = Fr[128]; C2[1:] = Fr[1:128]
    C3 = np.zeros((128, d)); C3[1:] = Fi[1:128]
    return np.concatenate([C0, C1, C2, C3], axis=1).astype(np.float32)  # [128, 4*D]


def host_consts(w1, b1, w2, b2, Wp, bp):
    w1r = np.asarray(w1, np.float64)[:, 0, :]   # (3, D)
    w2r = np.asarray(w2, np.float64)[:, 0, :]   # (128, D)
    Fm, Mi = _dft_consts()
    Cs = _spectral_weights(w2r)
    w1s = np.zeros((128, 3 * KD), dtype=np.float32)
    b1s = np.zeros((128, KD), dtype=np.float32)
    for k in range(KD):
        for j in range(3):
            w1s[:, j * KD + k] = w1r[j, k * 128:(k + 1) * 128]
        b1s[:, k] = np.asarray(b1, np.float64)[k * 128:(k + 1) * 128]
    WpT = np.ascontiguousarray(np.asarray(Wp, np.float32).T)      # [D, D]
    b2r = (NFFT * np.asarray(b2, np.float64)).astype(np.float32)[None, :]  # [1, D]
    bp1 = np.asarray(bp, np.float32)[None, :]                     # [1, D]
    eye = np.eye(128, dtype=np.float32)
    return dict(Fm=Fm, Minv=Mi, Cs=Cs, w1s=w1s, b1s=b1s, WpT=WpT,
                b2r=b2r, bp1=bp1, eye=eye)


def make_global_inputs(u, consts, put=None):
    """Global (already core-concatenated) input arrays for the PJRT runner.

    `put` (optional) asynchronously device_puts the constant arrays while the
    host threads quantize u, hiding the constants' upload entirely.
    """
    u = np.asarray(u, np.float32)
    g = {
        # shard-concats of the big consts are just the full matrices
        "WpT_sh": consts["WpT"], "Cs_sh": consts["Cs"],
        "Fm_sh": consts["Fm"], "Mi_sh": consts["Minv"],
        # small per-core consts, replicated
        "w1s": np.tile(consts["w1s"], (NCORES, 1)),
        "b1s": np.tile(consts["b1s"], (NCORES, 1)),
        "b2r": np.tile(consts["b2r"], (NCORES, 1)),
        "bp1": np.tile(consts["bp1"], (NCORES, 1)),
        "eye": np.tile(consts["eye"], (NCORES, 1)),
        "hmask": _HMASK,
    }
    if put is not None:
        g = {k: put(v) for k, v in g.items()}

    uf = u.reshape(B * L, D)
    if U_INT8:
        # quantize per core-band directly into the global buffer with a
        # small reused temp; odd cores' halos are copied from the previous
        # core's tail (same u rows, same quantization)
        if not _bufs:
            _bufs["t"] = np.empty((T_CORE, D), np.float32)
            _bufs["gu"] = np.zeros((NCORES * W_IN, D), np.int8)
            _bufs["gs"] = np.zeros((NCORES * W_IN, 1), np.float32)
        tbuf = _bufs["t"]
        g_u, g_us = _bufs["gu"], _bufs["gs"]
        for ci in range(NCORES):
            seg = uf[ci * T_CORE:(ci + 1) * T_CORE]
            rowmax = np.maximum(seg.max(axis=1), -seg.min(axis=1))
            inv = np.where(rowmax > 0, 127.0 / np.maximum(rowmax, 1e-30), 0.0)
            np.multiply(seg, inv[:, None], out=tbuf)
            np.rint(tbuf, out=tbuf)
            r0 = ci * W_IN + HALO
            g_u[r0:r0 + T_CORE] = tbuf
            g_us[r0:r0 + T_CORE, 0] = rowmax * (1.0 / 127.0)
        for ci in range(1, NCORES, 2):       # halo for mid-batch cores
            r0 = ci * W_IN
            g_u[r0:r0 + HALO] = g_u[r0 - HALO:r0]
            g_us[r0:r0 + HALO] = g_us[r0 - HALO:r0]
        g["u_tm"] = g_u
        g["u_sc"] = g_us
    else:
        g_u = np.zeros((NCORES * W_IN, D), BF)

        def mk_chunk(ci):
            bi, half = divmod(ci, NCORES // B)
            t0 = bi * L + half * T_CORE
            lo = max(bi * L, t0 - HALO)
            r0 = ci * W_IN + HALO - (t0 - lo)
            n = t0 + T_CORE - lo
            g_u[r0:r0 + n] = f32_to_bf16(uf[lo:lo + n])

        list(_pool.map(mk_chunk, range(NCORES)))
        g["u_tm"] = g_u
    return g


# ---------------------------------------------------------------- bass build
def build_nc(n_blocks=NB_FULL, mm_dt=MM_DT, reps=1):
    T = n_blocks * HOP
    W = HALO + T                       # uT width (2304 for full problem)
    nw = W // 128
    nc = bacc.Bacc("TRN2", target_bir_lowering=False, debug=False,
                   num_devices=NCORES)
    f32 = mybir.dt.float32
    bf16 = mybir.dt.bfloat16

    if U_INT8:
        u_d = nc.dram_tensor("u_tm", [W, D], mybir.dt.int8,
                             kind="ExternalInput").ap()
        us_d = nc.dram_tensor("u_sc", [W, 1], f32, kind="ExternalInput").ap()
    else:
        u_d = nc.dram_tensor("u_tm", [W, D], bf16, kind="ExternalInput").ap()
    WpT_d = nc.dram_tensor("WpT_sh", [128, D], f32, kind="ExternalInput").ap()
    Cs_d = nc.dram_tensor("Cs_sh", [16, 4 * D], f32, kind="ExternalInput").ap()
    Fm_d = nc.dram_tensor("Fm_sh", [16, 512], f32, kind="ExternalInput").ap()
    Mi_d = nc.dram_tensor("Mi_sh", [16, 256], f32, kind="ExternalInput").ap()
    w1s_d = nc.dram_tensor("w1s", [128, 3 * KD], f32, kind="ExternalInput").ap()
    b1s_d = nc.dram_tensor("b1s", [128, KD], f32, kind="ExternalInput").ap()
    b2r_d = nc.dram_tensor("b2r", [1, D], f32, kind="ExternalInput").ap()
    bp1_d = nc.dram_tensor("bp1", [1, D], f32, kind="ExternalInput").ap()
    eye_d = nc.dram_tensor("eye", [128, 128], f32, kind="ExternalInput").ap()
    hm_d = nc.dram_tensor("hmask", [128, 1], f32, kind="ExternalInput").ap()
    if Y_INT8:
        NG = D // YGRP
        # packed: [int8 data | bf16 scales bitcast to int8 pairs]
        yp_d = nc.dram_tensor("y_p", [T, D + 2 * NG], mybir.dt.int8,
                              kind="ExternalOutput").ap()
    else:
        y_d = nc.dram_tensor("y", [T, D], bf16, kind="ExternalOutput").ap()

    RG = [list(range(NCORES))]
    BYPASS = mybir.AluOpType.bypass

    from contextlib import ExitStack
    with tile.TileContext(nc) as tc, ExitStack() as ctx:
        dramp = ctx.enter_context(tc.tile_pool(name="ccdram", bufs=1,
                                               space="DRAM"))
        cpool = ctx.enter_context(tc.tile_pool(name="consts", bufs=1))

        # ---- AllGather the sharded constants over NeuronLink
        def gather(src_ap, rows, cols):
            bin_ = dramp.tile([rows, cols], f32)
            bout = dramp.tile([rows * NCORES, cols], f32)
            nc.gpsimd.dma_start(bin_[:], src_ap)
            nc.gpsimd.collective_compute(
                "AllGather", BYPASS, replica_groups=RG,
                ins=[bin_[:].opt()], outs=[bout[:].opt()])
            return bout

        wpt_g = gather(WpT_d[:], 128, D)        # [1024, 1024] = WpT
        cs_g = gather(Cs_d[:], 16, 4 * D)       # [128, 4096]
        fm_g = gather(Fm_d[:], 16, 512)         # [128, 512]
        mi_g = gather(Mi_d[:], 16, 256)         # [128, 256]

        # resident constants in SBUF
        wpt = cpool.tile([128, KD * D], f32, tag="wpt")
        for k in range(KD):
            nc.sync.dma_start(wpt[:, k * D:(k + 1) * D],
                              wpt_g[k * 128:(k + 1) * 128, :])
        fm = cpool.tile([128, 512], f32, tag="fm")
        nc.sync.dma_start(fm[:], fm_g[:])
        mi = cpool.tile([128, 256], f32, tag="mi")
        nc.sync.dma_start(mi[:], mi_g[:])
        cs = cpool.tile([128, 4 * D], f32, tag="cs")
        nc.sync.dma_start(cs[:], cs_g[:])
        w1s = cpool.tile([128, 3 * KD], f32, tag="w1s")
        nc.sync.dma_start(w1s[:], w1s_d[:])
        b1s = cpool.tile([128, KD], f32, tag="b1s")
        nc.sync.dma_start(b1s[:], b1s_d[:])
        b2r = cpool.tile([1, D], f32, tag="b2r")
        nc.sync.dma_start(b2r[:], b2r_d[:])
        bp1 = cpool.tile([1, D], f32, tag="bp1")
        nc.sync.dma_start(bp1[:], bp1_d[:])
        eye = cpool.tile([128, 128], f32, tag="eye")
        nc.sync.dma_start(eye[:], eye_d[:])
        hm = cpool.tile([128, 1], f32, tag="hm")
        nc.sync.dma_start(hm[:], hm_d[:])
        ones1 = cpool.tile([1, 128], f32, tag="ones1")
        nc.gpsimd.memset(ones1[:], 1.0)
        if not U_INT8:
            eye_bf = cpool.tile([128, 128], bf16, tag="eye_bf")
            nc.vector.tensor_copy(eye_bf[:], eye[:])

        # DRAM bounce for the channel-major u (written by the PE transposes)
        uT_b = dramp.tile([D, W], f32)
        uT3 = uT_b[:].rearrange("(k p) t -> p k t", p=128)

        utm_p = ctx.enter_context(tc.tile_pool(name="utm", bufs=3))
        upool = ctx.enter_context(tc.tile_pool(name="uq", bufs=3))
        scr = ctx.enter_context(tc.tile_pool(name="scr", bufs=6))
        hcm_p = ctx.enter_context(tc.tile_pool(name="hcm", bufs=2))
        hsb_p = ctx.enter_context(tc.tile_pool(name="hsb", bufs=3))
        yt_p = ctx.enter_context(tc.tile_pool(name="yt", bufs=4))
        psb_p = ctx.enter_context(tc.tile_pool(name="psb", bufs=4))
        ysb_p = ctx.enter_context(tc.tile_pool(name="ysb", bufs=2))
        gms_p = ctx.enter_context(tc.tile_pool(name="gms", bufs=2))

        htr_p = ctx.enter_context(tc.tile_pool(name="htr", bufs=1, space="PSUM"))
        xps_p = ctx.enter_context(tc.tile_pool(name="xps", bufs=1, space="PSUM"))
        vps_p = ctx.enter_context(tc.tile_pool(name="vps", bufs=2, space="PSUM"))
        pps_p = ctx.enter_context(tc.tile_pool(name="pps", bufs=2, space="PSUM"))

        MULT = mybir.AluOpType.mult
        ADD = mybir.AluOpType.add
        SILU = mybir.ActivationFunctionType.Silu
        COPY = mybir.ActivationFunctionType.Copy

        # ---- preamble: transpose time-major u into channel-major DRAM bounce
        # (PSUM from vps_p is free until the main loop's first IDFT)
        for w in range(nw):
            if U_INT8:
                ui = utm_p.tile([128, D], mybir.dt.int8, tag="utm8", bufs=2)
                nc.sync.dma_start(ui[:], u_d[w * 128:(w + 1) * 128, :])
                usc = scr.tile([128, 1], f32, tag="usc", bufs=2)
                nc.sync.dma_start(usc[:], us_d[w * 128:(w + 1) * 128, :])
                uf = utm_p.tile([128, D], f32, tag="utmf", bufs=2)
                nc.scalar.activation(uf[:], ui[:], COPY)
                ut = utm_p.tile([128, D], f32, tag="utms", bufs=2)
                nc.vector.tensor_scalar_mul(ut[:], uf[:], usc[:, 0:1])
                teye, tdt = eye, f32
            else:
                ub = utm_p.tile([128, D], bf16, tag="utmb", bufs=2)
                nc.sync.dma_start(ub[:], u_d[w * 128:(w + 1) * 128, :])
                ut = utm_p.tile([128, D], f32, tag="utmf", bufs=2)
                nc.scalar.activation(ut[:], ub[:], COPY)
                teye, tdt = eye, f32
            for g in range(2):
                ps = vps_p.tile([128, 512], f32, tag="vps")
                for j in range(4):
                    k = g * 4 + j
                    nc.tensor.transpose(
                        ps[:, j * 128:(j + 1) * 128],
                        ut[:, k * 128:(k + 1) * 128], teye[:])
                st = scr.tile([128, 512], f32, tag="pre_st", bufs=2)
                nc.scalar.activation(st[:], ps[:], COPY)
                for j in range(4):
                    k = g * 4 + j
                    nc.sync.dma_start(
                        uT_b[k * 128:(k + 1) * 128, w * 128:(w + 1) * 128],
                        st[:, j * 128:(j + 1) * 128])

        def mk_h_tile(hq):
            """conv1 (c-major, DVE+GPS) + silu (ACT) + transpose (PE) to a
            time-major h tile [128(t), D(ch)]."""
            base = HALO + hq * HOP
            uq = upool.tile([128, KD, 130], f32, tag="uq")
            nc.sync.dma_start(uq[:], uT3[:, :, base - 2:base + 128])
            hcm = hcm_p.tile([128, KD * 128], f32, tag="hcm")
            for k in range(KD):
                t1 = scr.tile([128, 128], f32, tag="scr1")
                nc.gpsimd.tensor_scalar(
                    t1[:], uq[:, k, 0:128], w1s[:, 0 * KD + k:0 * KD + k + 1],
                    None, MULT)
                t2 = scr.tile([128, 128], f32, tag="scr2")
                nc.gpsimd.tensor_scalar(
                    t2[:], uq[:, k, 1:129], w1s[:, 1 * KD + k:1 * KD + k + 1],
                    None, MULT)
                t3 = scr.tile([128, 128], f32, tag="scr3")
                nc.gpsimd.tensor_tensor(t3[:], t1[:], t2[:], ADD)
                t4 = scr.tile([128, 128], f32, tag="scr4")
                nc.vector.tensor_scalar(
                    t4[:], uq[:, k, 2:130], w1s[:, 2 * KD + k:2 * KD + k + 1],
                    b1s[:, k:k + 1], MULT, ADD)
                nc.vector.tensor_tensor(
                    hcm[:, k * 128:(k + 1) * 128], t3[:], t4[:], ADD)
            hcm2 = hcm_p.tile([128, KD * 128], f32, tag="hcm2")
            nc.scalar.activation(hcm2[:], hcm[:], SILU)
            htr = htr_p.tile([128, D], f32, tag="htr")
            for k in range(KD):
                nc.tensor.transpose(
                    htr[:, k * 128:(k + 1) * 128],
                    hcm2[:, k * 128:(k + 1) * 128], eye[:])
            hsb = hsb_p.tile([128, D], f32, tag="hsb")
            if hq < 0:
                nc.vector.tensor_scalar_mul(hsb[:], htr[:], hm[:, 0:1])
            else:
                nc.vector.tensor_copy(hsb[:], htr[:])
            return uq, hsb

        from contextlib import nullcontext
        loop_ctx = tc.For_i(0, reps, 1) if reps > 1 else nullcontext()
        with loop_ctx:
            h_tiles: dict = {}
            uq_tiles: dict = {}
            uq_tiles[-1], h_tiles[-1] = mk_h_tile(-1)
            uq_tiles[0], h_tiles[0] = mk_h_tile(0)
            for q in range(n_blocks):
                uq = uq_tiles.pop(q)
                hsb = h_tiles[q]
                hprev = h_tiles.pop(q - 1)
                ysb = ysb_p.tile([128, D], f32 if Y_INT8 else bf16, tag="ysb")
                # ---- GEMM both halves (PE work first; only needs uq + consts)
                pps_t = []
                for half in range(2):
                    e0 = half * 512
                    pps = pps_p.tile([128, 512], f32, tag="pps")
                    for k in range(KD):
                        nc.tensor.matmul(
                            pps[:],
                            uq[:, k, 2:130].bitcast(mm_dt),
                            wpt[:, k * D + e0:k * D + e0 + 512].bitcast(mm_dt),
                            start=(k == 0), stop=False)
                    nc.tensor.matmul(
                        pps[:], ones1[:].bitcast(mm_dt),
                        bp1[:, e0:e0 + 512].bitcast(mm_dt),
                        start=False, stop=True)
                    pps_t.append(pps)
                # ---- forward DFT both halves
                x_t = []
                for half in range(2):
                    e0 = half * 512
                    x0 = xps_p.tile([128, 512], f32, tag="xps0")
                    x1 = xps_p.tile([128, 512], f32, tag="xps1")
                    for bt, xps in ((0, x0), (1, x1)):
                        nc.tensor.matmul(
                            xps[:],
                            fm[:, (0 * 2 + bt) * 128:(0 * 2 + bt + 1) * 128].bitcast(mm_dt),
                            hprev[:, e0:e0 + 512].bitcast(mm_dt),
                            start=True, stop=False)
                        nc.tensor.matmul(
                            xps[:],
                            fm[:, (1 * 2 + bt) * 128:(1 * 2 + bt + 1) * 128].bitcast(mm_dt),
                            hsb[:, e0:e0 + 512].bitcast(mm_dt),
                            start=False, stop=True)
                    x_t.append((x0, x1))
                # ---- silu(p) early: frees GEMM PSUM banks a block sooner
                psb_t = []
                for half in range(2):
                    psb = psb_p.tile([128, 512], f32, tag="psb")
                    nc.scalar.activation(psb[:], pps_t[half][:], SILU)
                    psb_t.append(psb)
                # ---- spectral pointwise (DVE muls read PSUM; GPS does adds)
                yt_t = []
                for half in range(2):
                    e0 = half * 512
                    x0, x1 = x_t[half]
                    yt0 = yt_p.tile([128, 512], f32, tag="yt0")
                    yt1 = yt_p.tile([128, 512], f32, tag="yt1")
                    ta = scr.tile([128, 512], f32, tag="scra")
                    tb = scr.tile([128, 512], f32, tag="scrb")
                    nc.vector.tensor_tensor(yt0[:], x0[:], cs[:, 0 * D + e0:0 * D + e0 + 512], MULT)
                    nc.vector.tensor_tensor(ta[:], x1[:], cs[:, 1 * D + e0:1 * D + e0 + 512], MULT)
                    nc.gpsimd.tensor_tensor(yt0[:], yt0[:], ta[:], ADD)
                    nc.vector.tensor_tensor(
                        yt0[0:1, :], yt0[0:1, :], b2r[0:1, e0:e0 + 512], ADD)
                    nc.vector.tensor_tensor(yt1[:], x1[:], cs[:, 2 * D + e0:2 * D + e0 + 512], MULT)
                    nc.vector.tensor_tensor(tb[:], x0[:], cs[:, 3 * D + e0:3 * D + e0 + 512], MULT)
                    nc.gpsimd.tensor_tensor(yt1[:], yt1[:], tb[:], ADD)
                    yt_t.append((yt0, yt1))
                # ---- next block's h (PE transposes slot between DFT and IDFT,
                #      giving DVE/GPS time to finish pointwise)
                if q + 1 < n_blocks:
                    uq_tiles[q + 1], h_tiles[q + 1] = mk_h_tile(q + 1)
                # ---- inverse DFT + final multiply
                for half in range(2):
                    e0 = half * 512
                    yt0, yt1 = yt_t[half]
                    vps = vps_p.tile([128, 512], f32, tag="vps")
                    nc.tensor.matmul(vps[:], mi[:, 0:128].bitcast(mm_dt),
                                     yt0[:].bitcast(mm_dt), start=True, stop=False)
                    nc.tensor.matmul(vps[:], mi[:, 128:256].bitcast(mm_dt),
                                     yt1[:].bitcast(mm_dt), start=False, stop=True)
                    nc.vector.tensor_tensor(
                        ysb[:, e0:e0 + 512], vps[:], psb_t[half][:], MULT)
                if Y_INT8:
                    # quantize: q = rint(y * QMAX / bf16(groupmax)), scale out
                    NG = D // YGRP
                    yf3 = ysb[:].rearrange("p (g j) -> p g j", j=YGRP)
                    gm = gms_p.tile([128, NG], f32, tag="gm")
                    nc.vector.tensor_reduce(gm[:], yf3, mybir.AxisListType.X,
                                            mybir.AluOpType.max,
                                            apply_absolute_value=True)
                    gmb = gms_p.tile([128, NG], bf16, tag="gmb")
                    nc.gpsimd.tensor_copy(gmb[:], gm[:])
                    gmf = gms_p.tile([128, NG], f32, tag="gmf")
                    nc.gpsimd.tensor_copy(gmf[:], gmb[:])
                    inv = gms_p.tile([128, NG], f32, tag="inv")
                    nc.vector.reciprocal(inv[:], gmf[:])
                    invq = gms_p.tile([128, NG], f32, tag="invq")
                    nc.vector.tensor_scalar_mul(invq[:], inv[:], QMAX)
                    yq = ysb_p.tile([128, D], mybir.dt.int8, tag="yq")
                    yq3 = yq[:].rearrange("p (g j) -> p g j", j=YGRP)
                    for j in range(YGRP):
                        nc.vector.tensor_tensor(yq3[:, :, j], yf3[:, :, j],
                                                invq[:], MULT)
                    nc.sync.dma_start(yp_d[q * HOP:(q + 1) * HOP, 0:D], yq[:])
                    nc.sync.dma_start(yp_d[q * HOP:(q + 1) * HOP, D:D + 2 * NG],
                                      gmb[:].bitcast(mybir.dt.int8))
                else:
                    nc.sync.dma_start(y_d[q * HOP:(q + 1) * HOP, :], ysb[:])

    nc.compile()
    return nc


# ---------------------------------------------------------------- PJRT runner
def _make_runner(nc):
    """Cached jit callable mirroring bass2jax.run_bass_via_pjrt (multi-core)."""
    install_neuronx_cc_hook()
    partition_name = nc.partition_id_tensor.name if nc.partition_id_tensor else None
    in_names, out_names, out_avals, out_shapes = [], [], [], []
    for alloc in nc.m.functions[0].allocations:
        if not isinstance(alloc, mybir.MemoryLocationSet):
            continue
        name = alloc.memorylocations[0].name
        if alloc.kind == "ExternalInput":
            if name != partition_name:
                in_names.append(name)
        elif alloc.kind == "ExternalOutput":
            out_names.append(name)
            shape = tuple(alloc.tensor_shape)
            dtype = mybir.dt.np(alloc.dtype)
            out_avals.append(jax.core.ShapedArray(shape, dtype))
            out_shapes.append((shape, dtype))
    n_params = len(in_names)
    n_outs = len(out_avals)
    in_names_all = in_names + out_names
    if partition_name is not None:
        in_names_all.append(partition_name)
    donate = tuple(range(n_params, n_params + n_outs))

    def _body(*args):
        operands = list(args)
        if partition_name is not None:
            operands.append(partition_id_tensor())
        outs = _bass_exec_p.bind(
            *operands, out_avals=tuple(out_avals), in_names=tuple(in_names_all),
            out_names=tuple(out_names), lowering_input_output_aliases=(),
            sim_require_finite=True, sim_require_nnan=True, nc=nc)
        return tuple(outs)

    devices = jax.devices()[:NCORES]
    mesh = Mesh(np.asarray(devices), ("core",))
    in_specs = (PartitionSpec("core"),) * (n_params + n_outs)
    out_specs = (PartitionSpec("core"),) * len(out_names)
    # No donation: our NEFF writes every output element, so the zero buffers
    # are only parameter-order placeholders. Keeping them undonated lets us
    # upload them ONCE and reuse the device-resident copies every call.
    sharded = jax.jit(
        shard_map(_body, mesh=mesh, in_specs=in_specs, out_specs=out_specs,
                  check_rep=False),
        keep_unused=True)
    sharding = jax.sharding.NamedSharding(mesh, PartitionSpec("core"))
    dev_zeros: list = []

    def put(x):
        return jax.device_put(x, sharding)

    import os
    prof = bool(os.environ.get("KPROF"))

    def run(global_ins: dict):
        if not dev_zeros:
            dev_zeros.append(tuple(
                jax.device_put(np.zeros((NCORES * s[0], *s[1:]), dt), sharding)
                for s, dt in out_shapes))
        args = [global_ins[n] for n in in_names]
        if prof:
            import time
            t0 = time.perf_counter()
            outs = sharded(*args, *dev_zeros[0])
            t1 = time.perf_counter()
            jax.block_until_ready(outs)
            t2 = time.perf_counter()
            r = {}
            for i, n in enumerate(out_names):
                ta = time.perf_counter()
                r[n] = np.asarray(outs[i])
                print(f"  asarray[{n}] {r[n].nbytes/1e6:.1f}MB: "
                      f"{time.perf_counter()-ta:.3f}s")
            print(f"  dispatch={t1-t0:.3f} block={t2-t1:.3f}")
            return r
        outs = sharded(*args, *dev_zeros[0])
        return {n: np.asarray(outs[i]) for i, n in enumerate(out_names)}

    return run, put


def get_nc(n_blocks=NB_FULL, mm_dt=MM_DT, reps=1):
    key = (n_blocks, str(mm_dt), reps)
    if key not in _nc_cache:
        nc = build_nc(n_blocks, mm_dt, reps)
        run, put = _make_runner(nc)
        _nc_cache[key] = (nc, run, put)
    return _nc_cache[key]


# ---------------------------------------------------------------- entry point
_const_cache: dict = {}


def _weights_key(ws):
    import hashlib
    h = hashlib.blake2b(digest_size=16)
    for w in ws:
        a = np.ascontiguousarray(w)
        h.update(str(a.shape).encode())
        h.update(a.tobytes())
    return h.digest()


def kernel(u, w1, b1, w2, b2, Wp, bp):
    nc, run, put = get_nc()
    key = _weights_key([w1, b1, w2, b2, Wp, bp])
    if key not in _const_cache:
        consts = host_consts(w1, b1, w2, b2, Wp, bp)
        _const_cache.clear()   # keep at most one entry
        _const_cache[key] = consts
        put_consts = put       # device_put the const arrays (cached below)
    else:
        consts = _const_cache[key]
        put_consts = None      # already device-resident jax arrays
    gins = make_global_inputs(u, consts, put=put_consts)
    if put_consts is not None:
        # overwrite the host arrays with their device-resident versions so
        # the next call skips the upload entirely
        for k in ("WpT_sh", "Cs_sh", "Fm_sh", "Mi_sh", "w1s", "b1s", "b2r",
                  "bp1", "eye", "hmask"):
            consts.setdefault("_dev", {})[k] = gins[k]
    else:
        for k, v in consts.get("_dev", {}).items():
            gins[k] = v
    res = run(gins)
    y = np.empty((B, L, D), dtype=np.float32)

    if Y_INT8:
        NG = D // YGRP
        yp = res["y_p"]                # [NCORES*T_CORE, D + 2*NG] int8 packed

        def fill(ci):
            bi, half = divmod(ci, NCORES // B)
            sl = slice(ci * T_CORE, (ci + 1) * T_CORE)
            sb = np.ascontiguousarray(yp[sl, D:])          # [T, 2*NG] int8
            s32 = np.empty((T_CORE, NG), np.float32)
            bf16_to_f32(sb.view(BF), s32)
            s32 *= np.float32(1.0 / QMAX)
            out = y[bi, half * T_CORE:(half + 1) * T_CORE]
            np.multiply(yp[sl, :D].reshape(T_CORE, NG, YGRP), s32[:, :, None],
                        out=out.reshape(T_CORE, NG, YGRP))
    else:
        yg = res["y"]                  # [NCORES*T_CORE, D] bf16

        def fill(ci):
            bi, half = divmod(ci, NCORES // B)
            bf16_to_f32(yg[ci * T_CORE:(ci + 1) * T_CORE],
                        y[bi, half * T_CORE:(half + 1) * T_CORE])

    list(_pool.map(fill, range(NCORES)))
    return y
